# revision 26
# baseline (speedup 1.0000x reference)
"""Cross-attention kernel for Trainium2, distributed over 8 NeuronCores.

Sharding: data-parallel over batch (4) x tensor-parallel over head groups (2).
Core c handles batch b = c//2, heads [4g, 4g+4) with g = c%2.

Mask compaction: ~half the queries and ~half the keys are masked out by
`mask` / `context_mask`. The host compacts both before DMA:
  - only valid queries are sent (masked-query rows of the output are the
    uniform attention average, computed exactly on the host from column sums
    of context — a 2 MFLOP numpy job);
  - only valid keys are sent, with the null token at j slot 0 and zero
    padding up to a 128 multiple (padding columns get an additive -50 bias
    so exp ~ 2e-22, matching the reference's -inf masking to fp32 accuracy).
All matmul operands are bf16 (1 PE cycle/row vs 4 for fp32); PSUM
accumulation stays fp32.

Per-core device pipeline (layouts avoid on-device transposes; x^T/context^T
are produced host-side):
  qT  = tanh(Wq_g^T @ xc^T)                  [256, NIP]  (d on partitions)
  kT  = tanh(Wk_g^T @ cxc^T), null col 0     [256, NJP]
  v   = cxc @ Wv_g (+ null row, ones col)    [NJP, 4x65] (j on partitions)
  S^T = exp(0.125 * kT_h^T qT_h + padbias)   per (ichunk, jtile, head)
  outT_h = v_aug^T @ S^T  (row 64 = softmax denominator)
  divide by denominator, out_partial = O @ Wo_g  [NIP, 512]
Host sums the two head-group partials per batch, adds bo, scatters valid
rows, and fills masked-query rows with the uniform average.

PE instructions on TRN2 can carry at most ONE sync wait (walrus S3_LW /
ENGINE_NOP structs); Tile sometimes assigns more. `_split_pe_waits` runs
after scheduling and hoists extra waits onto PE nops inserted immediately
before the offending instruction — same engine stream, same blocking
semantics.
"""

import ml_dtypes
import numpy as np

import concourse.bass as bass
import concourse.tile as tile
from concourse import bacc, bass_utils, mybir

FP = mybir.dt.float32
BF = mybir.dt.bfloat16
NPBF = ml_dtypes.bfloat16
AF = mybir.ActivationFunctionType

B, N, M, DIM = 4, 2048, 2048, 512
HEADS, DH = 8, 64
G = 2          # head groups (tensor-parallel degree)
HG = 4         # heads per group
DG = HG * DH   # 256 dims per group
NEG = -50.0    # additive pad bias (exp(-50) ~ 2e-22)
SCALE = 1.0 / np.sqrt(DH)  # 0.125
VW = DH + 1    # v columns per head incl. ones column (den row)

LAST_RESULTS = None
_CACHE = {}


def _chunks(n, c):
    out = []
    while n > 0:
        out.append(min(c, n))
        n -= c
    return out


def _ichunks(nip):
    """i chunks, each 512..1024 wide (narrow chunks go latency-bound) and a
    512 last chunk so the final outproj tail stays short."""
    if nip <= 1024:
        return [nip]
    if nip <= 1536:
        return [nip - 512, 512]
    return [1024, nip - 1536, 512]


def _build(nip, njp):
    nc = bacc.Bacc("TRN2", debug=False, num_devices=8, enable_partition_id=False)
    d = {}
    njt = njp // 128

    def inp(name, shape, dt=BF):
        d[name] = nc.dram_tensor(name, shape, dt, kind="ExternalInput").ap()

    inp("xT", [DIM, nip])      # compacted valid queries, transposed
    inp("cxT", [DIM, njp])     # col 0 zero (null slot), compacted valid keys
    inp("wq", [DIM, DG])
    inp("wk", [DIM, DG])
    inp("wv", [DIM, DG])
    inp("wo", [DG, DIM])
    # col 0:njt = pad mask (1 for null+valid j, 0 for pad, partition-major);
    # col njt = null_key tiled x2
    inp("cmf", [128, njt + 1], FP)
    inp("nv", [1, HG * DH])     # null_value tiled x4
    d["out"] = nc.dram_tensor("out", [nip, DIM], BF, kind="ExternalOutput").ap()

    with tile.TileContext(nc) as tc:
        _body(tc, d, nip, njp)
    nc.compile()
    return nc


_SPLIT_SKIP = (
    "InstDrain", "InstUnconditionalBranch", "InstCall",
    "InstEventSemaphore", "InstRegisterMove", "InstDmaTrigger",
)


def _split_pe_waits(nc):
    """Hoist all-but-one sync waits from compute-engine instructions onto
    fresh same-engine nops placed immediately before them (TRN2 TPB
    instruction structs accept only one sync wait in walrus codegen;
    drains/branches/DMA handle waits differently)."""
    engines = {
        mybir.EngineType.PE: nc.tensor,
        mybir.EngineType.Activation: nc.scalar,
        mybir.EngineType.DVE: nc.vector,
        mybir.EngineType.Pool: nc.gpsimd,
        mybir.EngineType.SP: nc.sync,
    }
    total = 0
    for bb in nc.m.functions[0].blocks:
        new_insts = []
        for ins in bb.instructions:
            si = ins.sync_info
            eng = engines.get(getattr(ins, "engine", None))
            if (
                eng is not None
                and type(ins).__name__ not in _SPLIT_SKIP
                and si is not None
                and si.on_wait
                and len(si.on_wait) > 1
            ):
                waits = list(si.on_wait)
                for w in waits[:-1]:
                    nop = eng._isa(
                        nc.isa.Opcode.NEURON_ISA_TPB_OPCODE_ENGINE_NOP,
                        {}, None, [], [], True,
                    )
                    nop.sync_info = mybir.SyncInfo(on_wait=[w], on_update=[])
                    nc.inst_map[nop.name] = nop
                    new_insts.append(nop)
                    total += 1
                si.on_wait = waits[-1:]
            new_insts.append(ins)
        bb.instructions = new_insts
    return total


def _body(tc, d, nip, njp):
    nc = tc.nc
    njt = njp // 128
    nit = nip // 128

    with (
        tc.tile_pool(name="consts", bufs=1) as consts,
        tc.tile_pool(name="big", bufs=1) as big,
        tc.tile_pool(name="spool", bufs=5) as spool,
        tc.tile_pool(name="small", bufs=2) as small,
    ):
        # ---- inputs, ordered by first use (HWDGE is 625 ns per DMA instr,
        # serialized, so load order paces the start) ----
        xT = big.tile([128, 4, nip], BF)
        cxT = big.tile([128, 4, njp], BF)
        jh = min(512, njp)
        ich = _ichunks(nip)
        ih = ich[0]

        wk = consts.tile([128, 4, DG], BF)
        nc.sync.dma_start(wk[:], d["wk"].rearrange("(c p) d -> p c d", p=128))
        for c0 in (0, 2):   # cc pairs so the first matmuls start sooner
            nc.sync.dma_start(
                cxT[:, c0:c0 + 2, 0:jh],
                d["cxT"][c0 * 128:(c0 + 2) * 128, 0:jh]
                .rearrange("(c p) j -> p c j", p=128))
        wq = consts.tile([128, 4, DG], BF)
        nc.sync.dma_start(wq[:], d["wq"].rearrange("(c p) d -> p c d", p=128))
        for c0 in (0, 2):
            nc.sync.dma_start(
                xT[:, c0:c0 + 2, 0:ih],
                d["xT"][c0 * 128:(c0 + 2) * 128, 0:ih]
                .rearrange("(c p) i -> p c i", p=128))
        wv = consts.tile([128, 4, DG], BF)
        nc.sync.dma_start(wv[:], d["wv"].rearrange("(c p) d -> p c d", p=128))
        cmf = consts.tile([128, njt + 1], FP)
        nc.sync.dma_start(cmf[:], d["cmf"])
        nk = cmf[:, njt:njt + 1]
        if njp > jh:
            nc.sync.dma_start(
                cxT[:, :, jh:njp],
                d["cxT"][:, jh:njp].rearrange("(c p) j -> p c j", p=128))
        if nip > ih:
            nc.sync.dma_start(
                xT[:, :, ih:nip],
                d["xT"][:, ih:nip].rearrange("(c p) i -> p c i", p=128))
        # wo split in 64-row halves: outproj contracts 64 at a time so both
        # O halves can live on partitions 0:64 (no partition-shift DMAs)
        wo = consts.tile([DH, 4, DIM], BF)
        nc.sync.dma_start(wo[:], d["wo"].rearrange("(s p) o -> p s o", p=DH))

        negb = consts.tile([128, 1], FP)
        nc.vector.memset(negb[:], NEG)
        cmb = consts.tile([128, njt], FP)   # 0 where real j, NEG where pad
        nc.scalar.activation(cmb[:], cmf[:, 0:njt], AF.Identity, scale=-NEG,
                             bias=negb[:])

        ones_pd = consts.tile([128, DH], BF)
        nc.vector.memset(ones_pd[:], 1.0)

        qT = big.tile([128, 2, nip], BF)
        kT = big.tile([128, 2, njp], BF)
        vsb = big.tile([128, njt, HG, VW], BF)
        OsbL = big.tile([DH, 2, nip], BF)   # even heads (rows 0:64 per dc)
        OsbH = big.tile([DH, 2, nip], BF)   # odd heads (rows 64:128 per dc)

        # ---- one PSUM scope for everything: 4 + 4 banks ----
        with (
            tc.tile_pool(name="pss", bufs=2, space="PSUM") as pss_ps,
            tc.tile_pool(name="acc", bufs=2, space="PSUM") as acc_ps,
        ):
            def qproj(dc, i0, ch):
                ps = pss_ps.tile([128, 1024], FP, tag="pss", name=f"psq{dc}{i0}")
                for s0 in range(0, ch, 512):
                    sw = min(512, ch - s0)
                    for cc in range(4):
                        nc.tensor.matmul(
                            ps[:, s0:s0 + sw],
                            wq[:, cc, dc * 128:(dc + 1) * 128],
                            xT[:, cc, i0 + s0:i0 + s0 + sw],
                            start=(cc == 0), stop=(cc == 3),
                        )
                nc.scalar.activation(qT[:, dc, i0:i0 + ch], ps[:, 0:ch], AF.Tanh)

            def kproj(dc, j0):
                ch = min(512, njp - j0)
                ps = pss_ps.tile([128, 512], FP, tag="pss", name=f"psk{dc}{j0}")
                for cc in range(4):
                    nc.tensor.matmul(
                        ps[:, 0:ch],
                        wk[:, cc, dc * 128:(dc + 1) * 128],
                        cxT[:, cc, j0:j0 + ch],
                        start=(cc == 0), stop=(cc == 3),
                    )
                nc.scalar.activation(kT[:, dc, j0:j0 + ch], ps[:, 0:ch], AF.Tanh)

            def nulltanh(dc):
                nc.scalar.activation(kT[:, dc, 0:1], nk, AF.Tanh)

            def vproj(jt):
                ps = pss_ps.tile([128, DG], FP, tag="pss", name=f"psv{jt}")
                for cc in range(4):
                    nc.tensor.matmul(
                        ps[:],
                        cxT[:, cc, jt * 128:(jt + 1) * 128],
                        wv[:, cc, :],
                        start=(cc == 0), stop=(cc == 3),
                    )
                nc.vector.tensor_copy(
                    vsb[:, jt, :, 0:DH],
                    ps[:].rearrange("p (h e) -> p h e", h=HG),
                )
                nc.vector.memset(vsb[:, jt, :, DH:VW], 1.0)

            def outproj_mm(it):
                tsl = slice(it * 128, (it + 1) * 128)
                pf = pss_ps.tile([128, DIM], FP, tag="pss", name=f"pf{it}")
                for s, osb in enumerate((OsbL, OsbH, OsbL, OsbH)):
                    nc.tensor.matmul(
                        pf[:],
                        osb[0:DH, s // 2, tsl],
                        wo[:, s, :],
                        start=(s == 0), stop=(s == 3),
                    )
                return pf

            def outproj(it):
                pf = outproj_mm(it)
                fo = spool.tile([128, DIM], BF, tag="fo", name=f"fo{it}")
                nc.vector.tensor_copy(fo[:], pf[:])   # Pool can't read PSUM
                nc.sync.dma_start(d["out"][it * 128:(it + 1) * 128, :], fo[:])

            # minimal upfront work: only what S(block 0, jt 0) and PV(0) need
            kproj(0, 0)
            nulltanh(0)
            vproj(0)
            qproj(0, 0, ich[0])
            # null token (j = 0) overwrites the zero row the projection made
            nc.sync.dma_start(vsb[0:1, 0, :, 0:DH],
                              d["nv"].rearrange("a (h e) -> a h e", h=HG))

            # everything else drips into free jt slots, ordered by the last
            # slot it may be emitted at (deadline = just-in-time emission;
            # drips always run before the next S so the PE never outruns them)
            drip = []
            for k in range(1, njt):
                drip.append(((0, max(0, k - 1), 1), lambda k=k: vproj(k)))
            for j0 in range(512, njp, 512):
                drip.append(((0, j0 // 128 - 1, 0),
                             lambda j0=j0: kproj(0, j0)))
            drip.append(((1, max(0, njt - 3), 0), lambda: kproj(1, 0)))
            drip.append(((1, max(0, njt - 3), 0), lambda: nulltanh(1)))
            for j0 in range(512, njp, 512):
                drip.append(((2, j0 // 128 - 1, 0),
                             lambda j0=j0: kproj(1, j0)))
            i0 = 0
            for ci, ch in enumerate(ich):
                bq = 4 * ci + 1   # dc1 of chunk ci first used in block 4ci+2
                drip.append(((bq, max(0, njt - 3), 0),
                             lambda i0=i0, ch=ch: qproj(1, i0, ch)))
                if ci > 0:
                    drip.append(((4 * ci - 1, max(0, njt - 3), 0),
                                 lambda i0=i0, ch=ch: qproj(0, i0, ch)))
                i0 += ch
            drip.sort(key=lambda e: e[0])

            def s_mm(h, i0, ch, jt):
                prow, dc = 64 * (h % 2), h // 2
                pss = pss_ps.tile([128, 1024], FP, tag="pss",
                                  name=f"pss{h}{i0}{jt}")
                for s0 in range(0, ch, 512):
                    sw = min(512, ch - s0)
                    nc.tensor.matmul(
                        pss[:, s0:s0 + sw],
                        kT[prow:prow + DH, dc, jt * 128:(jt + 1) * 128],
                        qT[prow:prow + DH, dc, i0 + s0:i0 + s0 + sw],
                        start=True, stop=True,
                    )
                return pss

            # flash blocks: chunk-outer so a finished chunk's outproj can be
            # dripped into later blocks; divide-tails deferred one block
            blocks = []
            i0 = 0
            for ch in _ichunks(nip):
                blocks += [(i0, ch, h) for h in range(HG)]
                i0 += ch
            pending = []     # divide-tail closures from the previous block
            outq = []        # (append_bi, it) for deferred outproj tiles

            po_cur = acc_ps.tile([128, 1024], FP, tag="po", name="po_first")
            pss_cur = s_mm(blocks[0][2], blocks[0][0], blocks[0][1], 0)
            for bi, (i0, ch, h) in enumerate(blocks):
                dc = h // 2
                po, pss = po_cur, pss_cur
                for jt in range(njt):
                    Ssb = spool.tile([128, 1024], BF, tag="s",
                                     name=f"s{h}{i0}{jt}")
                    nc.scalar.activation(Ssb[:, 0:ch], pss[:, 0:ch], AF.Exp,
                                         bias=cmb[:, jt:jt + 1],
                                         scale=float(SCALE))
                    # dripped projections (before the next S emission)
                    dripped = False
                    while drip and drip[0][0][:2] <= (bi, jt):
                        drip.pop(0)[1]()
                        dripped = True
                    if not dripped and drip and jt not in (2, 4, 6):
                        drip.pop(0)[1]()
                    if jt + 1 < njt:
                        pss = s_mm(h, i0, ch, jt + 1)
                    if jt == max(0, njt - 2) and bi + 1 < len(blocks):
                        # pre-emit the next block's accumulator + first S one
                        # iter early so its exp follows our last exp directly
                        ni0, nch, nh = blocks[bi + 1]
                        po_cur = acc_ps.tile([128, 1024], FP, tag="po",
                                             name=f"po{nh}{ni0}")
                        pss_cur = s_mm(nh, ni0, nch, 0)
                    if jt == min(2, njt - 1):
                        # flush the previous block's divide-tail mid-loop so
                        # its pr matmul never stalls the PE behind the recip
                        for fn in pending:
                            fn()
                        pending = []
                    elif jt in (4, 6) and outq and outq[0][0] + 1 <= bi:
                        outproj(outq.pop(0)[1])
                    for s0 in range(0, ch, 512):
                        sw = min(512, ch - s0)
                        nc.tensor.matmul(
                            po[0:VW, s0:s0 + sw],
                            vsb[:, jt, h, :],
                            Ssb[:, s0:s0 + sw],
                            start=(jt == 0), stop=(jt == njt - 1),
                        )
                denR = small.tile([128, 1024], BF, tag="den", name=f"dr{h}{i0}")
                with nc.allow_low_precision(
                        reason="1/den in bf16; rel-err budget is 2e-2"):
                    nc.vector.reciprocal(denR[DH:VW, 0:ch], po[DH:VW, 0:ch])

                def tail(po=po, denR=denR, h=h, dc=dc, i0=i0, ch=ch,
                         pieces=1, after_piece=None):
                    osb = OsbL if h % 2 == 0 else OsbH
                    pr = pss_ps.tile([DH, 1024], FP, tag="pss",
                                     name=f"pr{h}{i0}")
                    prs = small.tile([DH, 1024], BF, tag="prs",
                                     name=f"pb{h}{i0}")
                    pw = ch // pieces
                    for p0 in range(0, ch, pw):
                        for s0 in range(p0, p0 + pw, 512):
                            sw = min(512, p0 + pw - s0)
                            nc.tensor.matmul(pr[:, s0:s0 + sw],
                                             ones_pd[DH:VW, 0:DH],
                                             denR[DH:VW, s0:s0 + sw],
                                             start=True, stop=True)
                        nc.vector.tensor_copy(prs[:, p0:p0 + pw],
                                              pr[:, p0:p0 + pw])
                        nc.vector.tensor_mul(
                            osb[0:DH, dc, i0 + p0:i0 + p0 + pw],
                            po[0:DH, p0:p0 + pw], prs[:, p0:p0 + pw])
                        if after_piece is not None:
                            after_piece(i0 + p0)

                pending.append(tail)
                if h == HG - 1:
                    outq += [(bi, it) for it in
                             range(i0 // 128, (i0 + ch) // 128)]

            # final tail: the last block's divide runs in 128-wide pieces and
            # each piece immediately feeds its outproj tile; copies alternate
            # DVE/Act, halves ship as soon as their copies land
            assert len(pending) == 1 and not drip
            tail_tiles = [it for _, it in outq]
            fi0, fch, fh = blocks[-1]
            if tail_tiles:
                t0 = tail_tiles[0]
                nt = len(tail_tiles)
                assert tail_tiles == list(range(t0, t0 + nt))
                fo_all = spool.tile([128, nt, DIM], BF,
                                    tag="fotail", name="fo_tail")
                copies = [nc.vector.tensor_copy, nc.scalar.copy]
                half = (nt + 1) // 2

                def emit_tile(it):
                    idx = it - t0
                    pf = outproj_mm(it)
                    copies[idx % 2](fo_all[:, idx, :], pf[:])
                    if idx + 1 in (half, nt):
                        g0, g1 = (0, half) if idx + 1 == half else (half, nt)
                        nc.sync.dma_start(
                            d["out"][(t0 + g0) * 128:(t0 + g1) * 128, :]
                            .rearrange("(t p) o -> p t o", p=128),
                            fo_all[:, g0:g1, :])

                # tiles from earlier chunks (rare) are already divided
                for it in tail_tiles:
                    if it * 128 < fi0:
                        emit_tile(it)

                def after_piece(a0):
                    it = a0 // 128
                    if it in tail_tiles:
                        emit_tile(it)

                pending[0](pieces=max(1, fch // 128),
                           after_piece=after_piece)
            else:
                pending[0](pieces=max(1, fch // 128))
            pending = []


def _pad128(n):
    return max(128, (n + 127) & ~127)


def _core_inputs(inputs, core, nip, njp):
    b, g = core // 2, core % 2
    x = np.asarray(inputs["x"], np.float32)
    context = np.asarray(inputs["context"], np.float32)
    mask = np.asarray(inputs["mask"])
    context_mask = np.asarray(inputs["context_mask"])
    Wq = np.asarray(inputs["Wq"], np.float32)
    Wkv = np.asarray(inputs["Wkv"], np.float32)
    Wo = np.asarray(inputs["Wo"], np.float32)
    null_key = np.asarray(inputs["null_key"], np.float32)
    null_value = np.asarray(inputs["null_value"], np.float32)
    njt = njp // 128

    vi = np.flatnonzero(mask[b])
    vj = np.flatnonzero(context_mask[b])
    xc = np.zeros((nip, DIM), np.float32)
    xc[:len(vi)] = x[b][vi]
    cxc = np.zeros((njp, DIM), np.float32)
    cxc[1:1 + len(vj)] = context[b][vj]
    cm = np.zeros(njp, np.float32)
    cm[:1 + len(vj)] = 1.0
    cmf = np.concatenate(
        [cm.reshape(njt, 128).T, np.tile(null_key, 2).reshape(128, 1)], axis=1)

    gs = slice(g * DG, (g + 1) * DG)
    return {
        "xT": np.ascontiguousarray(xc.T).astype(NPBF),
        "cxT": np.ascontiguousarray(cxc.T).astype(NPBF),
        "wq": np.ascontiguousarray(Wq[:, gs]).astype(NPBF),
        "wk": np.ascontiguousarray(Wkv[:, gs]).astype(NPBF),
        "wv": np.ascontiguousarray(
            Wkv[:, DIM + g * DG: DIM + (g + 1) * DG]).astype(NPBF),
        "wo": np.ascontiguousarray(Wo[gs, :]).astype(NPBF),
        "cmf": np.ascontiguousarray(cmf),
        "nv": np.tile(null_value, HG).reshape(1, HG * DH).astype(NPBF),
    }


def kernel(x, context, mask, context_mask, Wq, Wkv, Wo, bo, null_key, null_value):
    global LAST_RESULTS
    inputs = {
        "x": x, "context": context, "mask": mask, "context_mask": context_mask,
        "Wq": Wq, "Wkv": Wkv, "Wo": Wo, "bo": bo,
        "null_key": null_key, "null_value": null_value,
    }
    mask = np.asarray(mask)
    context_mask = np.asarray(context_mask)
    nip = _pad128(int(mask.sum(1).max()))
    njp = _pad128(int(context_mask.sum(1).max()) + 1)

    key = (nip, njp)
    if key not in _CACHE:
        _CACHE[key] = _build(nip, njp)
        _CACHE["nc"] = _CACHE[key]   # convenience handle for test.py
    nc = _CACHE[key]
    in_maps = [_core_inputs(inputs, core, nip, njp) for core in range(8)]
    res = bass_utils.run_bass_kernel_spmd(nc, in_maps, core_ids=list(range(8)))
    LAST_RESULTS = res

    x_np = np.asarray(x, np.float32)
    ctx_np = np.asarray(context, np.float32)
    Wkv_np = np.asarray(Wkv, np.float32)
    Wo_np = np.asarray(Wo, np.float32)
    bo_np = np.asarray(bo, np.float32)
    nv_np = np.asarray(null_value, np.float32)

    out = np.empty((B, N, DIM), np.float32)
    for b in range(B):
        vi = np.flatnonzero(mask[b])
        dev = (np.asarray(res.results[2 * b]["out"], np.float32)
               + np.asarray(res.results[2 * b + 1]["out"], np.float32))
        out[b][vi] = dev[:len(vi)] + bo_np
        if len(vi) < N:
            # masked queries: uniform attention over [null, all keys]
            vsum = ctx_np[b].sum(0) @ Wkv_np[:, DIM:] + np.tile(nv_np, HEADS)
            urow = (vsum / (M + 1)) @ Wo_np + bo_np
            out[b][~mask[b]] = urow
    return out


# revision 46
# speedup vs baseline: 1.0259x; 1.0259x over previous
"""Cross-attention kernel for Trainium2, distributed over 8 NeuronCores.

Sharding: data-parallel over batch (4) x tensor-parallel over head groups (2).
Core c handles batch b = c//2, heads [4g, 4g+4) with g = c%2.

Mask compaction: ~half the queries and ~half the keys are masked out by
`mask` / `context_mask`. The host compacts both before DMA:
  - only valid queries are sent (masked-query rows of the output are the
    uniform attention average, computed exactly on the host from column sums
    of context — a 2 MFLOP numpy job);
  - only valid keys are sent, with the null token at j slot 0 and zero
    padding up to a 128 multiple (padding columns get an additive -50 bias
    so exp ~ 2e-22, matching the reference's -inf masking to fp32 accuracy).
All matmul operands are bf16 (1 PE cycle/row vs 4 for fp32); PSUM
accumulation stays fp32.

Per-core device pipeline (layouts avoid on-device transposes; x^T/context^T
are produced host-side):
  qT  = tanh(Wq_g^T @ xc^T)                  [256, NIP]  (d on partitions)
  kT  = tanh(Wk_g^T @ cxc^T), null col 0     [256, NJP]
  v   = cxc @ Wv_g (+ null row, ones col)    [NJP, 4x65] (j on partitions)
  S^T = exp(0.125 * kT_h^T qT_h + padbias)   per (ichunk, jtile, head)
  outT_h = v_aug^T @ S^T  (row 64 = softmax denominator)
  divide by denominator, out_partial = O @ Wo_g  [NIP, 512]
Host sums the two head-group partials per batch, adds bo, scatters valid
rows, and fills masked-query rows with the uniform average.

PE instructions on TRN2 can carry at most ONE sync wait (walrus S3_LW /
ENGINE_NOP structs); Tile sometimes assigns more. `_split_pe_waits` runs
after scheduling and hoists extra waits onto PE nops inserted immediately
before the offending instruction — same engine stream, same blocking
semantics.
"""

import ml_dtypes
import numpy as np

import concourse.bass as bass
import concourse.tile as tile
from concourse import bacc, bass_utils, mybir

FP = mybir.dt.float32
BF = mybir.dt.bfloat16
NPBF = ml_dtypes.bfloat16
AF = mybir.ActivationFunctionType

B, N, M, DIM = 4, 2048, 2048, 512
HEADS, DH = 8, 64
G = 2          # head groups (tensor-parallel degree)
HG = 4         # heads per group
DG = HG * DH   # 256 dims per group
NEG = -50.0    # additive pad bias (exp(-50) ~ 2e-22)
SCALE = 1.0 / np.sqrt(DH)  # 0.125
VW = DH + 1    # v columns per head incl. ones column (den row)

LAST_RESULTS = None
_CACHE = {}


def _chunks(n, c):
    out = []
    while n > 0:
        out.append(min(c, n))
        n -= c
    return out


def _ichunks(nip):
    """i chunks, each 512..1024 wide (narrow chunks go latency-bound) and a
    512 last chunk so the final outproj tail stays short."""
    if nip <= 1024:
        return [nip]
    if nip <= 1536:
        return [nip - 512, 512]
    return [1024, nip - 1536, 512]


def _build(nip, njp):
    nc = bacc.Bacc("TRN2", debug=False, num_devices=8, enable_partition_id=False)
    d = {}
    njt = njp // 128

    def inp(name, shape, dt=BF):
        d[name] = nc.dram_tensor(name, shape, dt, kind="ExternalInput").ap()

    inp("xT", [DIM, nip])      # compacted valid queries, transposed
    inp("cxT", [DIM, njp])     # col 0 zero (null slot), compacted valid keys
    inp("wq", [DIM, DG])
    inp("wk", [DIM, DG])
    inp("wv", [DIM, DG])
    inp("wo", [DG, DIM])
    # col 0:njt = pad mask (1 for null+valid j, 0 for pad, partition-major);
    # col njt = null_key tiled x2
    inp("cmf", [128, njt + 1], FP)
    inp("nv", [1, HG * DH])     # null_value tiled x4
    d["out"] = nc.dram_tensor("out", [nip, DIM], BF, kind="ExternalOutput").ap()

    with tile.TileContext(nc) as tc:
        _body(tc, d, nip, njp)
    nc.compile()
    return nc


_SPLIT_SKIP = (
    "InstDrain", "InstUnconditionalBranch", "InstCall",
    "InstEventSemaphore", "InstRegisterMove", "InstDmaTrigger",
)


def _split_pe_waits(nc):
    """Hoist all-but-one sync waits from compute-engine instructions onto
    fresh same-engine nops placed immediately before them (TRN2 TPB
    instruction structs accept only one sync wait in walrus codegen;
    drains/branches/DMA handle waits differently)."""
    engines = {
        mybir.EngineType.PE: nc.tensor,
        mybir.EngineType.Activation: nc.scalar,
        mybir.EngineType.DVE: nc.vector,
        mybir.EngineType.Pool: nc.gpsimd,
        mybir.EngineType.SP: nc.sync,
    }
    total = 0
    for bb in nc.m.functions[0].blocks:
        new_insts = []
        for ins in bb.instructions:
            si = ins.sync_info
            eng = engines.get(getattr(ins, "engine", None))
            if (
                eng is not None
                and type(ins).__name__ not in _SPLIT_SKIP
                and si is not None
                and si.on_wait
                and len(si.on_wait) > 1
            ):
                waits = list(si.on_wait)
                for w in waits[:-1]:
                    nop = eng._isa(
                        nc.isa.Opcode.NEURON_ISA_TPB_OPCODE_ENGINE_NOP,
                        {}, None, [], [], True,
                    )
                    nop.sync_info = mybir.SyncInfo(on_wait=[w], on_update=[])
                    nc.inst_map[nop.name] = nop
                    new_insts.append(nop)
                    total += 1
                si.on_wait = waits[-1:]
            new_insts.append(ins)
        bb.instructions = new_insts
    return total


def _body(tc, d, nip, njp):
    nc = tc.nc
    njt = njp // 128
    nit = nip // 128

    with (
        tc.tile_pool(name="consts", bufs=1) as consts,
        tc.tile_pool(name="big", bufs=1) as big,
        tc.tile_pool(name="spool", bufs=5) as spool,
        tc.tile_pool(name="small", bufs=2) as small,
    ):
        # ---- inputs, ordered by first use (HWDGE is 625 ns per DMA instr,
        # serialized, so load order paces the start) ----
        xT = big.tile([128, 4, nip], BF)
        cxT = big.tile([128, 4, njp], BF)
        jh = min(512, njp)
        ich = _ichunks(nip)
        ih = ich[0]

        warm = consts.tile([DH, 512], BF)
        nc.vector.memset(warm[:], 0.0)   # first DVE op: feeds the PE warmup

        wk = consts.tile([128, 4, DG], BF)
        nc.sync.dma_start(wk[:], d["wk"].rearrange("(c p) d -> p c d", p=128))
        for c0 in (0, 2):   # cc pairs so the first matmuls start sooner
            nc.sync.dma_start(
                cxT[:, c0:c0 + 2, 0:jh],
                d["cxT"][c0 * 128:(c0 + 2) * 128, 0:jh]
                .rearrange("(c p) j -> p c j", p=128))
        wq = consts.tile([128, 4, DG], BF)
        nc.sync.dma_start(wq[:], d["wq"].rearrange("(c p) d -> p c d", p=128))
        for c0 in (0, 2):
            nc.sync.dma_start(
                xT[:, c0:c0 + 2, 0:ih],
                d["xT"][c0 * 128:(c0 + 2) * 128, 0:ih]
                .rearrange("(c p) i -> p c i", p=128))
        wv = consts.tile([128, 4, DG], BF)
        nc.sync.dma_start(wv[:], d["wv"].rearrange("(c p) d -> p c d", p=128))
        cmf = consts.tile([128, njt + 1], FP)
        nc.sync.dma_start(cmf[:], d["cmf"])
        nk = cmf[:, njt:njt + 1]
        if njp > jh:
            nc.sync.dma_start(
                cxT[:, :, jh:njp],
                d["cxT"][:, jh:njp].rearrange("(c p) j -> p c j", p=128))
        if nip > ih:
            nc.sync.dma_start(
                xT[:, :, ih:nip],
                d["xT"][:, ih:nip].rearrange("(c p) i -> p c i", p=128))
        # wo split in 64-row halves: outproj contracts 64 at a time so both
        # O halves can live on partitions 0:64 (no partition-shift DMAs)
        wo = consts.tile([DH, 4, DIM], BF)
        nc.sync.dma_start(wo[:], d["wo"].rearrange("(s p) o -> p s o", p=DH))

        negb = consts.tile([128, 1], FP)
        nc.vector.memset(negb[:], NEG)
        cmb = consts.tile([128, njt], FP)   # 0 where real j, NEG where pad
        nc.scalar.activation(cmb[:], cmf[:, 0:njt], AF.Identity, scale=-NEG,
                             bias=negb[:])

        ones_pd = consts.tile([128, DH], BF)
        nc.vector.memset(ones_pd[:], 1.0)

        qT = big.tile([128, 2, nip], BF)
        kT = big.tile([128, 2, njp], BF)
        vsb = big.tile([128, njt, HG, VW], BF)
        OsbL = big.tile([DH, 2, nip], BF)   # even heads (rows 0:64 per dc)
        OsbH = big.tile([DH, 2, nip], BF)   # odd heads (rows 64:128 per dc)

        # ---- one PSUM scope for everything: 4 + 4 banks ----
        with (
            tc.tile_pool(name="pss", bufs=2, space="PSUM") as pss_ps,
            tc.tile_pool(name="acc", bufs=2, space="PSUM") as acc_ps,
        ):
            def qproj(dc, i0, ch):
                ps = pss_ps.tile([128, 1024], FP, tag="pss", name=f"psq{dc}{i0}")
                for s0 in range(0, ch, 512):
                    sw = min(512, ch - s0)
                    for cc in range(4):
                        nc.tensor.matmul(
                            ps[:, s0:s0 + sw],
                            wq[:, cc, dc * 128:(dc + 1) * 128],
                            xT[:, cc, i0 + s0:i0 + s0 + sw],
                            start=(cc == 0), stop=(cc == 3),
                        )
                nc.scalar.activation(qT[:, dc, i0:i0 + ch], ps[:, 0:ch], AF.Tanh)

            def kproj(dc, j0):
                ch = min(512, njp - j0)
                ps = pss_ps.tile([128, 512], FP, tag="pss", name=f"psk{dc}{j0}")
                for cc in range(4):
                    nc.tensor.matmul(
                        ps[:, 0:ch],
                        wk[:, cc, dc * 128:(dc + 1) * 128],
                        cxT[:, cc, j0:j0 + ch],
                        start=(cc == 0), stop=(cc == 3),
                    )
                nc.scalar.activation(kT[:, dc, j0:j0 + ch], ps[:, 0:ch], AF.Tanh)

            def nulltanh(dc):
                nc.scalar.activation(kT[:, dc, 0:1], nk, AF.Tanh)

            def vproj(jt):
                ps = pss_ps.tile([128, DG], FP, tag="pss", name=f"psv{jt}")
                for cc in range(4):
                    nc.tensor.matmul(
                        ps[:],
                        cxT[:, cc, jt * 128:(jt + 1) * 128],
                        wv[:, cc, :],
                        start=(cc == 0), stop=(cc == 3),
                    )
                nc.vector.tensor_copy(
                    vsb[:, jt, :, 0:DH],
                    ps[:].rearrange("p (h e) -> p h e", h=HG),
                )
                nc.vector.memset(vsb[:, jt, :, DH:VW], 1.0)

            def outproj_mm(it):
                tsl = slice(it * 128, (it + 1) * 128)
                pf = pss_ps.tile([128, DIM], FP, tag="pss", name=f"pf{it}")
                for s, osb in enumerate((OsbL, OsbH, OsbL, OsbH)):
                    nc.tensor.matmul(
                        pf[:],
                        osb[0:DH, s // 2, tsl],
                        wo[:, s, :],
                        start=(s == 0), stop=(s == 3),
                    )
                return pf

            def outproj(it):
                pf = outproj_mm(it)
                fo = spool.tile([128, DIM], BF, tag="fo", name=f"fo{it}")
                nc.vector.tensor_copy(fo[:], pf[:])   # Pool can't read PSUM
                nc.sync.dma_start(d["out"][it * 128:(it + 1) * 128, :], fo[:])

            # p-state warmup: ~3 us of dependency-free matmuls while the
            # first input DMAs land, so the real projections start at the
            # full 2.4 GHz clock instead of the cold 0.65/1.2 GHz tiers
            wps = pss_ps.tile([128, 1024], FP, tag="pss", name="warmps")
            for r in range(9):
                nc.tensor.matmul(wps[0:DH, 0:512], warm[:, 0:DH],
                                 warm[:, 0:512], start=True, stop=True)

            # minimal upfront work: only what S(block 0, jt 0) needs (vproj
            # is dripped — its wv arrives after wk/wq, and PV(0) runs a full
            # exp later than S(0))
            kproj(0, 0)
            nulltanh(0)
            qproj(0, 0, ich[0])

            def vproj0():
                vproj(0)
                # null token (j = 0) overwrites the projection's zero row
                nc.sync.dma_start(vsb[0:1, 0, :, 0:DH],
                                  d["nv"].rearrange("a (h e) -> a h e", h=HG))

            # everything else drips into free jt slots, ordered by the last
            # slot it may be emitted at (deadline = just-in-time emission;
            # drips always run before the next S so the PE never outruns them)
            drip = [((0, 0, 0), vproj0)]
            for k in range(1, njt):
                drip.append(((0, max(0, k - 1), 1), lambda k=k: vproj(k)))
            for j0 in range(512, njp, 512):
                drip.append(((0, j0 // 128 - 1, 0),
                             lambda j0=j0: kproj(0, j0)))
            drip.append(((1, max(0, njt - 3), 0), lambda: kproj(1, 0)))
            drip.append(((1, max(0, njt - 3), 0), lambda: nulltanh(1)))
            for j0 in range(512, njp, 512):
                drip.append(((2, j0 // 128 - 1, 0),
                             lambda j0=j0: kproj(1, j0)))
            i0 = 0
            for ci, ch in enumerate(ich):
                bq = 4 * ci + 1   # dc1 of chunk ci first used in block 4ci+2
                drip.append(((bq, max(0, njt - 3), 0),
                             lambda i0=i0, ch=ch: qproj(1, i0, ch)))
                if ci > 0:
                    drip.append(((4 * ci - 1, max(0, njt - 3), 0),
                                 lambda i0=i0, ch=ch: qproj(0, i0, ch)))
                i0 += ch
            drip.sort(key=lambda e: e[0])

            def s_mm(h, i0, ch, jt):
                prow, dc = 64 * (h % 2), h // 2
                pss = pss_ps.tile([128, 1024], FP, tag="pss",
                                  name=f"pss{h}{i0}{jt}")
                for s0 in range(0, ch, 512):
                    sw = min(512, ch - s0)
                    nc.tensor.matmul(
                        pss[:, s0:s0 + sw],
                        kT[prow:prow + DH, dc, jt * 128:(jt + 1) * 128],
                        qT[prow:prow + DH, dc, i0 + s0:i0 + s0 + sw],
                        start=True, stop=True,
                    )
                return pss

            # flash blocks: chunk-outer so a finished chunk's outproj can be
            # dripped into later blocks; divide-tails deferred one block
            blocks = []
            i0 = 0
            for ch in _ichunks(nip):
                blocks += [(i0, ch, h) for h in range(HG)]
                i0 += ch
            pending = []     # divide-tail closures from the previous block
            outq = []        # (append_bi, it) for deferred outproj tiles

            po_cur = acc_ps.tile([128, 1024], FP, tag="po", name="po_first")
            pss_cur = s_mm(blocks[0][2], blocks[0][0], blocks[0][1], 0)
            for bi, (i0, ch, h) in enumerate(blocks):
                dc = h // 2
                po, pss = po_cur, pss_cur
                for jt in range(njt):
                    Ssb = spool.tile([128, 1024], BF, tag="s",
                                     name=f"s{h}{i0}{jt}")
                    nc.scalar.activation(Ssb[:, 0:ch], pss[:, 0:ch], AF.Exp,
                                         bias=cmb[:, jt:jt + 1],
                                         scale=float(SCALE))
                    # dripped projections (before the next S emission)
                    dripped = False
                    while drip and drip[0][0][:2] <= (bi, jt):
                        drip.pop(0)[1]()
                        dripped = True
                    if not dripped and drip and jt not in (2, 4, 6):
                        drip.pop(0)[1]()
                    if jt + 1 < njt:
                        pss = s_mm(h, i0, ch, jt + 1)
                    if jt == max(0, njt - 2) and bi + 1 < len(blocks):
                        # pre-emit the next block's accumulator + first S one
                        # iter early so its exp follows our last exp directly
                        ni0, nch, nh = blocks[bi + 1]
                        po_cur = acc_ps.tile([128, 1024], FP, tag="po",
                                             name=f"po{nh}{ni0}")
                        pss_cur = s_mm(nh, ni0, nch, 0)
                    if jt == min(2, njt - 1):
                        # flush the previous block's divide-tail mid-loop so
                        # its pr matmul never stalls the PE behind the recip
                        for fn in pending:
                            fn()
                        pending = []
                    elif jt in (4, 6) and outq and outq[0][0] + 1 <= bi:
                        outproj(outq.pop(0)[1])
                    for s0 in range(0, ch, 512):
                        sw = min(512, ch - s0)
                        nc.tensor.matmul(
                            po[0:VW, s0:s0 + sw],
                            vsb[:, jt, h, :],
                            Ssb[:, s0:s0 + sw],
                            start=(jt == 0), stop=(jt == njt - 1),
                        )
                denR = small.tile([128, 1024], BF, tag="den", name=f"dr{h}{i0}")
                with nc.allow_low_precision(
                        reason="1/den in bf16; rel-err budget is 2e-2"):
                    nc.vector.reciprocal(denR[DH:VW, 0:ch], po[DH:VW, 0:ch])

                def tail(po=po, denR=denR, h=h, dc=dc, i0=i0, ch=ch,
                         pieces=1, after_piece=None):
                    osb = OsbL if h % 2 == 0 else OsbH
                    pr = pss_ps.tile([DH, 1024], FP, tag="pss",
                                     name=f"pr{h}{i0}")
                    prs = small.tile([DH, 1024], BF, tag="prs",
                                     name=f"pb{h}{i0}")
                    pw = ch // pieces
                    for p0 in range(0, ch, pw):
                        for s0 in range(p0, p0 + pw, 512):
                            sw = min(512, p0 + pw - s0)
                            nc.tensor.matmul(pr[:, s0:s0 + sw],
                                             ones_pd[DH:VW, 0:DH],
                                             denR[DH:VW, s0:s0 + sw],
                                             start=True, stop=True)
                        nc.vector.tensor_copy(prs[:, p0:p0 + pw],
                                              pr[:, p0:p0 + pw])
                        nc.vector.tensor_mul(
                            osb[0:DH, dc, i0 + p0:i0 + p0 + pw],
                            po[0:DH, p0:p0 + pw], prs[:, p0:p0 + pw])
                        if after_piece is not None:
                            after_piece(i0 + p0)

                pending.append(tail)
                if h == HG - 1:
                    outq += [(bi, it) for it in
                             range(i0 // 128, (i0 + ch) // 128)]

            # final tail: the last block's divide runs in 128-wide pieces and
            # each piece immediately feeds its outproj tile; copies alternate
            # DVE/Act, halves ship as soon as their copies land
            assert len(pending) == 1 and not drip
            tail_tiles = [it for _, it in outq]
            fi0, fch, fh = blocks[-1]
            if tail_tiles:
                t0 = tail_tiles[0]
                nt = len(tail_tiles)
                assert tail_tiles == list(range(t0, t0 + nt))
                fo_all = spool.tile([128, nt, DIM], BF,
                                    tag="fotail", name="fo_tail")
                # Act is idle once the last exp retires; keep the DVE free
                # for the reciprocal/divide pieces that gate each tile
                copies = [nc.scalar.copy, nc.scalar.copy]
                half = (nt + 1) // 2

                def emit_tile(it):
                    idx = it - t0
                    pf = outproj_mm(it)
                    copies[idx % 2](fo_all[:, idx, :], pf[:])
                    # per-tile DMAs: the HWDGE stage pipelines under the
                    # remaining copies, so the last transfer starts sooner
                    nc.sync.dma_start(
                        d["out"][it * 128:(it + 1) * 128, :],
                        fo_all[:, idx, :])

                # tiles from earlier chunks (rare) are already divided
                for it in tail_tiles:
                    if it * 128 < fi0:
                        emit_tile(it)

                def after_piece(a0):
                    it = a0 // 128
                    if it in tail_tiles:
                        emit_tile(it)

                pending[0](pieces=max(1, fch // 128),
                           after_piece=after_piece)
            else:
                pending[0](pieces=max(1, fch // 128))
            pending = []


def _pad128(n):
    return max(128, (n + 127) & ~127)


def _core_inputs(inputs, core, nip, njp):
    b, g = core // 2, core % 2
    x = np.asarray(inputs["x"], np.float32)
    context = np.asarray(inputs["context"], np.float32)
    mask = np.asarray(inputs["mask"])
    context_mask = np.asarray(inputs["context_mask"])
    Wq = np.asarray(inputs["Wq"], np.float32)
    Wkv = np.asarray(inputs["Wkv"], np.float32)
    Wo = np.asarray(inputs["Wo"], np.float32)
    null_key = np.asarray(inputs["null_key"], np.float32)
    null_value = np.asarray(inputs["null_value"], np.float32)
    njt = njp // 128

    vi = np.flatnonzero(mask[b])
    vj = np.flatnonzero(context_mask[b])
    xc = np.zeros((nip, DIM), np.float32)
    xc[:len(vi)] = x[b][vi]
    cxc = np.zeros((njp, DIM), np.float32)
    cxc[1:1 + len(vj)] = context[b][vj]
    cm = np.zeros(njp, np.float32)
    cm[:1 + len(vj)] = 1.0
    cmf = np.concatenate(
        [cm.reshape(njt, 128).T, np.tile(null_key, 2).reshape(128, 1)], axis=1)

    gs = slice(g * DG, (g + 1) * DG)
    return {
        "xT": np.ascontiguousarray(xc.T).astype(NPBF),
        "cxT": np.ascontiguousarray(cxc.T).astype(NPBF),
        "wq": np.ascontiguousarray(Wq[:, gs]).astype(NPBF),
        "wk": np.ascontiguousarray(Wkv[:, gs]).astype(NPBF),
        "wv": np.ascontiguousarray(
            Wkv[:, DIM + g * DG: DIM + (g + 1) * DG]).astype(NPBF),
        "wo": np.ascontiguousarray(Wo[gs, :]).astype(NPBF),
        "cmf": np.ascontiguousarray(cmf),
        "nv": np.tile(null_value, HG).reshape(1, HG * DH).astype(NPBF),
    }


def kernel(x, context, mask, context_mask, Wq, Wkv, Wo, bo, null_key, null_value):
    global LAST_RESULTS
    inputs = {
        "x": x, "context": context, "mask": mask, "context_mask": context_mask,
        "Wq": Wq, "Wkv": Wkv, "Wo": Wo, "bo": bo,
        "null_key": null_key, "null_value": null_value,
    }
    mask = np.asarray(mask)
    context_mask = np.asarray(context_mask)
    nip = _pad128(int(mask.sum(1).max()))
    njp = _pad128(int(context_mask.sum(1).max()) + 1)

    key = (nip, njp)
    if key not in _CACHE:
        _CACHE[key] = _build(nip, njp)
        _CACHE["nc"] = _CACHE[key]   # convenience handle for test.py
    nc = _CACHE[key]
    in_maps = [_core_inputs(inputs, core, nip, njp) for core in range(8)]
    res = bass_utils.run_bass_kernel_spmd(nc, in_maps, core_ids=list(range(8)))
    LAST_RESULTS = res

    x_np = np.asarray(x, np.float32)
    ctx_np = np.asarray(context, np.float32)
    Wkv_np = np.asarray(Wkv, np.float32)
    Wo_np = np.asarray(Wo, np.float32)
    bo_np = np.asarray(bo, np.float32)
    nv_np = np.asarray(null_value, np.float32)

    out = np.empty((B, N, DIM), np.float32)
    for b in range(B):
        vi = np.flatnonzero(mask[b])
        dev = (np.asarray(res.results[2 * b]["out"], np.float32)
               + np.asarray(res.results[2 * b + 1]["out"], np.float32))
        out[b][vi] = dev[:len(vi)] + bo_np
        if len(vi) < N:
            # masked queries: uniform attention over [null, all keys]
            vsum = ctx_np[b].sum(0) @ Wkv_np[:, DIM:] + np.tile(nv_np, HEADS)
            urow = (vsum / (M + 1)) @ Wo_np + bo_np
            out[b][~mask[b]] = urow
    return out


# revision 48
# speedup vs baseline: 1.0307x; 1.0046x over previous
"""Cross-attention kernel for Trainium2, distributed over 8 NeuronCores.

Sharding: data-parallel over batch (4) x tensor-parallel over head groups (2).
Core c handles batch b = c//2, heads [4g, 4g+4) with g = c%2.

Mask compaction: ~half the queries and ~half the keys are masked out by
`mask` / `context_mask`. The host compacts both before DMA:
  - only valid queries are sent (masked-query rows of the output are the
    uniform attention average, computed exactly on the host from column sums
    of context — a 2 MFLOP numpy job);
  - only valid keys are sent, with the null token at j slot 0 and zero
    padding up to a 128 multiple (padding columns get an additive -50 bias
    so exp ~ 2e-22, matching the reference's -inf masking to fp32 accuracy).
All matmul operands are bf16 (1 PE cycle/row vs 4 for fp32); PSUM
accumulation stays fp32.

Per-core device pipeline (layouts avoid on-device transposes; x^T/context^T
are produced host-side):
  qT  = tanh(Wq_g^T @ xc^T)                  [256, NIP]  (d on partitions)
  kT  = tanh(Wk_g^T @ cxc^T), null col 0     [256, NJP]
  v   = cxc @ Wv_g (+ null row, ones col)    [NJP, 4x65] (j on partitions)
  S^T = exp(0.125 * kT_h^T qT_h + padbias)   per (ichunk, jtile, head)
  outT_h = v_aug^T @ S^T  (row 64 = softmax denominator)
  divide by denominator, out_partial = O @ Wo_g  [NIP, 512]
Host sums the two head-group partials per batch, adds bo, scatters valid
rows, and fills masked-query rows with the uniform average.

PE instructions on TRN2 can carry at most ONE sync wait (walrus S3_LW /
ENGINE_NOP structs); Tile sometimes assigns more. `_split_pe_waits` runs
after scheduling and hoists extra waits onto PE nops inserted immediately
before the offending instruction — same engine stream, same blocking
semantics.
"""

import ml_dtypes
import numpy as np

import concourse.bass as bass
import concourse.tile as tile
from concourse import bacc, bass_utils, mybir

FP = mybir.dt.float32
BF = mybir.dt.bfloat16
NPBF = ml_dtypes.bfloat16
AF = mybir.ActivationFunctionType

B, N, M, DIM = 4, 2048, 2048, 512
HEADS, DH = 8, 64
G = 2          # head groups (tensor-parallel degree)
HG = 4         # heads per group
DG = HG * DH   # 256 dims per group
NEG = -50.0    # additive pad bias (exp(-50) ~ 2e-22)
SCALE = 1.0 / np.sqrt(DH)  # 0.125
VW = DH + 1    # v columns per head incl. ones column (den row)

LAST_RESULTS = None
_CACHE = {}


def _chunks(n, c):
    out = []
    while n > 0:
        out.append(min(c, n))
        n -= c
    return out


def _ichunks(nip):
    """i chunks, each 512..1024 wide (narrow chunks go latency-bound) and a
    512 last chunk so the final outproj tail stays short."""
    if nip <= 1024:
        return [nip]
    if nip <= 1536:
        return [nip - 512, 512]
    return [1024, nip - 1536, 512]


def _build(nip, njp):
    nc = bacc.Bacc("TRN2", debug=False, num_devices=8, enable_partition_id=False)
    d = {}
    njt = njp // 128

    def inp(name, shape, dt=BF):
        d[name] = nc.dram_tensor(name, shape, dt, kind="ExternalInput").ap()

    inp("xT", [DIM, nip])      # compacted valid queries, transposed
    inp("cxT", [DIM, njp])     # col 0 zero (null slot), compacted valid keys
    inp("wq", [DIM, DG])
    inp("wk", [DIM, DG])
    inp("wv", [DIM, DG])
    inp("wo", [DG, DIM])
    # col 0:njt = pad mask (1 for null+valid j, 0 for pad, partition-major);
    # col njt = null_key tiled x2
    inp("cmf", [128, njt + 1], FP)
    inp("nv", [1, HG * DH])     # null_value tiled x4
    d["out"] = nc.dram_tensor("out", [nip, DIM], BF, kind="ExternalOutput").ap()

    with tile.TileContext(nc) as tc:
        _body(tc, d, nip, njp)
    nc.compile()
    return nc


_SPLIT_SKIP = (
    "InstDrain", "InstUnconditionalBranch", "InstCall",
    "InstEventSemaphore", "InstRegisterMove", "InstDmaTrigger",
)


def _split_pe_waits(nc):
    """Hoist all-but-one sync waits from compute-engine instructions onto
    fresh same-engine nops placed immediately before them (TRN2 TPB
    instruction structs accept only one sync wait in walrus codegen;
    drains/branches/DMA handle waits differently)."""
    engines = {
        mybir.EngineType.PE: nc.tensor,
        mybir.EngineType.Activation: nc.scalar,
        mybir.EngineType.DVE: nc.vector,
        mybir.EngineType.Pool: nc.gpsimd,
        mybir.EngineType.SP: nc.sync,
    }
    total = 0
    for bb in nc.m.functions[0].blocks:
        new_insts = []
        for ins in bb.instructions:
            si = ins.sync_info
            eng = engines.get(getattr(ins, "engine", None))
            if (
                eng is not None
                and type(ins).__name__ not in _SPLIT_SKIP
                and si is not None
                and si.on_wait
                and len(si.on_wait) > 1
            ):
                waits = list(si.on_wait)
                for w in waits[:-1]:
                    nop = eng._isa(
                        nc.isa.Opcode.NEURON_ISA_TPB_OPCODE_ENGINE_NOP,
                        {}, None, [], [], True,
                    )
                    nop.sync_info = mybir.SyncInfo(on_wait=[w], on_update=[])
                    nc.inst_map[nop.name] = nop
                    new_insts.append(nop)
                    total += 1
                si.on_wait = waits[-1:]
            new_insts.append(ins)
        bb.instructions = new_insts
    return total


def _body(tc, d, nip, njp):
    nc = tc.nc
    njt = njp // 128
    nit = nip // 128

    with (
        tc.tile_pool(name="consts", bufs=1) as consts,
        tc.tile_pool(name="big", bufs=1) as big,
        tc.tile_pool(name="spool", bufs=5) as spool,
        tc.tile_pool(name="small", bufs=2) as small,
    ):
        # ---- inputs, ordered by first use (HWDGE is 625 ns per DMA instr,
        # serialized, so load order paces the start) ----
        xT = big.tile([128, 4, nip], BF)
        cxT = big.tile([128, 4, njp], BF)
        jh = min(512, njp)
        ich = _ichunks(nip)
        ih = ich[0]

        warm = consts.tile([DH, 512], BF)
        nc.vector.memset(warm[:], 0.0)   # first DVE op: feeds the PE warmup

        wk = consts.tile([128, 4, DG], BF)
        nc.sync.dma_start(wk[:], d["wk"].rearrange("(c p) d -> p c d", p=128))
        for c0 in (0, 2):   # cc pairs so the first matmuls start sooner
            nc.sync.dma_start(
                cxT[:, c0:c0 + 2, 0:jh],
                d["cxT"][c0 * 128:(c0 + 2) * 128, 0:jh]
                .rearrange("(c p) j -> p c j", p=128))
        wq = consts.tile([128, 4, DG], BF)
        nc.sync.dma_start(wq[:], d["wq"].rearrange("(c p) d -> p c d", p=128))
        for c0 in (0, 2):
            nc.sync.dma_start(
                xT[:, c0:c0 + 2, 0:ih],
                d["xT"][c0 * 128:(c0 + 2) * 128, 0:ih]
                .rearrange("(c p) i -> p c i", p=128))
        wv = consts.tile([128, 4, DG], BF)
        nc.sync.dma_start(wv[:], d["wv"].rearrange("(c p) d -> p c d", p=128))
        cmf = consts.tile([128, njt + 1], FP)
        nc.sync.dma_start(cmf[:], d["cmf"])
        nk = cmf[:, njt:njt + 1]
        if njp > jh:
            nc.sync.dma_start(
                cxT[:, :, jh:njp],
                d["cxT"][:, jh:njp].rearrange("(c p) j -> p c j", p=128))
        if nip > ih:
            nc.sync.dma_start(
                xT[:, :, ih:nip],
                d["xT"][:, ih:nip].rearrange("(c p) i -> p c i", p=128))
        # wo split in 64-row halves: outproj contracts 64 at a time so both
        # O halves can live on partitions 0:64 (no partition-shift DMAs)
        wo = consts.tile([DH, 4, DIM], BF)
        nc.sync.dma_start(wo[:], d["wo"].rearrange("(s p) o -> p s o", p=DH))

        negb = consts.tile([128, 1], FP)
        nc.vector.memset(negb[:], NEG)
        cmb = consts.tile([128, njt], FP)   # 0 where real j, NEG where pad
        nc.scalar.activation(cmb[:], cmf[:, 0:njt], AF.Identity, scale=-NEG,
                             bias=negb[:])

        ones_pd = consts.tile([128, DH], BF)
        nc.vector.memset(ones_pd[:], 1.0)

        qT = big.tile([128, 2, nip], BF)
        kT = big.tile([128, 2, njp], BF)
        vsb = big.tile([128, njt, HG, VW], BF)
        OsbL = big.tile([DH, 2, nip], BF)   # even heads (rows 0:64 per dc)
        OsbH = big.tile([DH, 2, nip], BF)   # odd heads (rows 64:128 per dc)

        # ---- one PSUM scope for everything: 4 + 4 banks ----
        with (
            tc.tile_pool(name="pss", bufs=2, space="PSUM") as pss_ps,
            tc.tile_pool(name="acc", bufs=2, space="PSUM") as acc_ps,
        ):
            def qproj(dc, i0, ch):
                ps = pss_ps.tile([128, 1024], FP, tag="pss", name=f"psq{dc}{i0}")
                for s0 in range(0, ch, 512):
                    sw = min(512, ch - s0)
                    for cc in range(4):
                        nc.tensor.matmul(
                            ps[:, s0:s0 + sw],
                            wq[:, cc, dc * 128:(dc + 1) * 128],
                            xT[:, cc, i0 + s0:i0 + s0 + sw],
                            start=(cc == 0), stop=(cc == 3),
                        )
                nc.scalar.activation(qT[:, dc, i0:i0 + ch], ps[:, 0:ch], AF.Tanh)

            def kproj(dc, j0):
                ch = min(512, njp - j0)
                ps = pss_ps.tile([128, 512], FP, tag="pss", name=f"psk{dc}{j0}")
                for cc in range(4):
                    nc.tensor.matmul(
                        ps[:, 0:ch],
                        wk[:, cc, dc * 128:(dc + 1) * 128],
                        cxT[:, cc, j0:j0 + ch],
                        start=(cc == 0), stop=(cc == 3),
                    )
                nc.scalar.activation(kT[:, dc, j0:j0 + ch], ps[:, 0:ch], AF.Tanh)

            def nulltanh(dc):
                nc.scalar.activation(kT[:, dc, 0:1], nk, AF.Tanh)

            def vproj(jt):
                ps = pss_ps.tile([128, DG], FP, tag="pss", name=f"psv{jt}")
                for cc in range(4):
                    nc.tensor.matmul(
                        ps[:],
                        cxT[:, cc, jt * 128:(jt + 1) * 128],
                        wv[:, cc, :],
                        start=(cc == 0), stop=(cc == 3),
                    )
                nc.vector.tensor_copy(
                    vsb[:, jt, :, 0:DH],
                    ps[:].rearrange("p (h e) -> p h e", h=HG),
                )
                nc.vector.memset(vsb[:, jt, :, DH:VW], 1.0)

            def outproj_mm(it):
                tsl = slice(it * 128, (it + 1) * 128)
                pf = pss_ps.tile([128, DIM], FP, tag="pss", name=f"pf{it}")
                for s, osb in enumerate((OsbL, OsbH, OsbL, OsbH)):
                    nc.tensor.matmul(
                        pf[:],
                        osb[0:DH, s // 2, tsl],
                        wo[:, s, :],
                        start=(s == 0), stop=(s == 3),
                    )
                return pf

            def outproj(it):
                pf = outproj_mm(it)
                fo = spool.tile([128, DIM], BF, tag="fo", name=f"fo{it}")
                nc.vector.tensor_copy(fo[:], pf[:])   # Pool can't read PSUM
                nc.sync.dma_start(d["out"][it * 128:(it + 1) * 128, :], fo[:])

            # p-state warmup: ~3 us of dependency-free matmuls while the
            # first input DMAs land, so the real projections start at the
            # full 2.4 GHz clock instead of the cold 0.65/1.2 GHz tiers
            wps = pss_ps.tile([128, 1024], FP, tag="pss", name="warmps")
            for r in range(9):
                nc.tensor.matmul(wps[0:DH, 0:512], warm[:, 0:DH],
                                 warm[:, 0:512], start=True, stop=True)

            # minimal upfront work: only what S(block 0, jt 0) needs (vproj
            # is dripped — its wv arrives after wk/wq, and PV(0) runs a full
            # exp later than S(0))
            kproj(0, 0)
            nulltanh(0)
            qproj(0, 0, ich[0])

            def vproj0():
                vproj(0)
                # null token (j = 0) overwrites the projection's zero row
                nc.sync.dma_start(vsb[0:1, 0, :, 0:DH],
                                  d["nv"].rearrange("a (h e) -> a h e", h=HG))

            # everything else drips into free jt slots, ordered by the last
            # slot it may be emitted at (deadline = just-in-time emission;
            # drips always run before the next S so the PE never outruns them)
            drip = [((0, 0, 0), vproj0)]
            for k in range(1, njt):
                drip.append(((0, max(0, k - 1), 1), lambda k=k: vproj(k)))
            for j0 in range(512, njp, 512):
                drip.append(((0, j0 // 128 - 1, 0),
                             lambda j0=j0: kproj(0, j0)))
            drip.append(((1, max(0, njt - 3), 0), lambda: kproj(1, 0)))
            drip.append(((1, max(0, njt - 3), 0), lambda: nulltanh(1)))
            for j0 in range(512, njp, 512):
                drip.append(((2, j0 // 128 - 1, 0),
                             lambda j0=j0: kproj(1, j0)))
            i0 = 0
            for ci, ch in enumerate(ich):
                bq = 4 * ci + 1   # dc1 of chunk ci first used in block 4ci+2
                drip.append(((bq, max(0, njt - 3), 0),
                             lambda i0=i0, ch=ch: qproj(1, i0, ch)))
                if ci > 0:
                    drip.append(((4 * ci - 1, max(0, njt - 3), 0),
                                 lambda i0=i0, ch=ch: qproj(0, i0, ch)))
                i0 += ch
            drip.sort(key=lambda e: e[0])

            def s_mm(h, i0, ch, jt):
                prow, dc = 64 * (h % 2), h // 2
                pss = pss_ps.tile([128, 1024], FP, tag="pss",
                                  name=f"pss{h}{i0}{jt}")
                for s0 in range(0, ch, 512):
                    sw = min(512, ch - s0)
                    nc.tensor.matmul(
                        pss[:, s0:s0 + sw],
                        kT[prow:prow + DH, dc, jt * 128:(jt + 1) * 128],
                        qT[prow:prow + DH, dc, i0 + s0:i0 + s0 + sw],
                        start=True, stop=True,
                    )
                return pss

            # flash blocks: chunk-outer so a finished chunk's outproj can be
            # dripped into later blocks; divide-tails deferred one block
            blocks = []
            i0 = 0
            for ch in _ichunks(nip):
                blocks += [(i0, ch, h) for h in range(HG)]
                i0 += ch
            pending = []     # divide-tail closures from the previous block
            outq = []        # (append_bi, it) for deferred outproj tiles

            po_cur = acc_ps.tile([128, 1024], FP, tag="po", name="po_first")
            pss_cur = s_mm(blocks[0][2], blocks[0][0], blocks[0][1], 0)
            for bi, (i0, ch, h) in enumerate(blocks):
                dc = h // 2
                po, pss = po_cur, pss_cur
                for jt in range(njt):
                    Ssb = spool.tile([128, 1024], BF, tag="s",
                                     name=f"s{h}{i0}{jt}")
                    nc.scalar.activation(Ssb[:, 0:ch], pss[:, 0:ch], AF.Exp,
                                         bias=cmb[:, jt:jt + 1],
                                         scale=float(SCALE))
                    # dripped projections (before the next S emission)
                    dripped = False
                    while drip and drip[0][0][:2] <= (bi, jt):
                        drip.pop(0)[1]()
                        dripped = True
                    if not dripped and drip and jt not in (2, 4, 6):
                        drip.pop(0)[1]()
                    if jt + 1 < njt:
                        pss = s_mm(h, i0, ch, jt + 1)
                    if jt == max(0, njt - 2) and bi + 1 < len(blocks):
                        # pre-emit the next block's accumulator + first S one
                        # iter early so its exp follows our last exp directly
                        ni0, nch, nh = blocks[bi + 1]
                        po_cur = acc_ps.tile([128, 1024], FP, tag="po",
                                             name=f"po{nh}{ni0}")
                        pss_cur = s_mm(nh, ni0, nch, 0)
                    if jt == min(2, njt - 1):
                        # flush the previous block's divide-tail mid-loop so
                        # its pr matmul never stalls the PE behind the recip
                        for fn in pending:
                            fn()
                        pending = []
                    elif jt in (4, 6) and outq and outq[0][0] + 1 <= bi:
                        outproj(outq.pop(0)[1])
                    for s0 in range(0, ch, 512):
                        sw = min(512, ch - s0)
                        nc.tensor.matmul(
                            po[0:VW, s0:s0 + sw],
                            vsb[:, jt, h, :],
                            Ssb[:, s0:s0 + sw],
                            start=(jt == 0), stop=(jt == njt - 1),
                        )
                denR = small.tile([128, 1024], BF, tag="den", name=f"dr{h}{i0}")
                with nc.allow_low_precision(
                        reason="1/den in bf16; rel-err budget is 2e-2"):
                    nc.vector.reciprocal(denR[DH:VW, 0:ch], po[DH:VW, 0:ch])

                def tail(po=po, denR=denR, h=h, dc=dc, i0=i0, ch=ch,
                         pieces=1, after_piece=None):
                    osb = OsbL if h % 2 == 0 else OsbH
                    pr = pss_ps.tile([DH, 1024], FP, tag="pss",
                                     name=f"pr{h}{i0}")
                    prs = small.tile([DH, 1024], BF, tag="prs",
                                     name=f"pb{h}{i0}")
                    pw = ch // pieces
                    for p0 in range(0, ch, pw):
                        for s0 in range(p0, p0 + pw, 512):
                            sw = min(512, p0 + pw - s0)
                            nc.tensor.matmul(pr[:, s0:s0 + sw],
                                             ones_pd[DH:VW, 0:DH],
                                             denR[DH:VW, s0:s0 + sw],
                                             start=True, stop=True)
                        nc.vector.tensor_copy(prs[:, p0:p0 + pw],
                                              pr[:, p0:p0 + pw])
                        nc.vector.tensor_mul(
                            osb[0:DH, dc, i0 + p0:i0 + p0 + pw],
                            po[0:DH, p0:p0 + pw], prs[:, p0:p0 + pw])
                        if after_piece is not None:
                            for t0 in range(p0, p0 + pw, 128):
                                after_piece(i0 + t0)

                pending.append(tail)
                if h == HG - 1:
                    outq += [(bi, it) for it in
                             range(i0 // 128, (i0 + ch) // 128)]

            # final tail: the last block's divide runs in 128-wide pieces and
            # each piece immediately feeds its outproj tile; copies alternate
            # DVE/Act, halves ship as soon as their copies land
            assert len(pending) == 1 and not drip
            tail_tiles = [it for _, it in outq]
            fi0, fch, fh = blocks[-1]
            if tail_tiles:
                t0 = tail_tiles[0]
                nt = len(tail_tiles)
                assert tail_tiles == list(range(t0, t0 + nt))
                fo_all = spool.tile([128, nt, DIM], BF,
                                    tag="fotail", name="fo_tail")
                # Act is idle once the last exp retires; keep the DVE free
                # for the reciprocal/divide pieces that gate each tile
                copies = [nc.scalar.copy, nc.scalar.copy]
                half = (nt + 1) // 2

                def emit_tile(it):
                    idx = it - t0
                    pf = outproj_mm(it)
                    copies[idx % 2](fo_all[:, idx, :], pf[:])
                    # per-tile DMAs: the HWDGE stage pipelines under the
                    # remaining copies, so the last transfer starts sooner
                    nc.sync.dma_start(
                        d["out"][it * 128:(it + 1) * 128, :],
                        fo_all[:, idx, :])

                # tiles from earlier chunks (rare) are already divided
                for it in tail_tiles:
                    if it * 128 < fi0:
                        emit_tile(it)

                def after_piece(a0):
                    it = a0 // 128
                    if it in tail_tiles:
                        emit_tile(it)

                pending[0](pieces=max(1, fch // 256),
                           after_piece=after_piece)
            else:
                pending[0](pieces=max(1, fch // 256))
            pending = []


def _pad128(n):
    return max(128, (n + 127) & ~127)


def _core_inputs(inputs, core, nip, njp):
    b, g = core // 2, core % 2
    x = np.asarray(inputs["x"], np.float32)
    context = np.asarray(inputs["context"], np.float32)
    mask = np.asarray(inputs["mask"])
    context_mask = np.asarray(inputs["context_mask"])
    Wq = np.asarray(inputs["Wq"], np.float32)
    Wkv = np.asarray(inputs["Wkv"], np.float32)
    Wo = np.asarray(inputs["Wo"], np.float32)
    null_key = np.asarray(inputs["null_key"], np.float32)
    null_value = np.asarray(inputs["null_value"], np.float32)
    njt = njp // 128

    vi = np.flatnonzero(mask[b])
    vj = np.flatnonzero(context_mask[b])
    xc = np.zeros((nip, DIM), np.float32)
    xc[:len(vi)] = x[b][vi]
    cxc = np.zeros((njp, DIM), np.float32)
    cxc[1:1 + len(vj)] = context[b][vj]
    cm = np.zeros(njp, np.float32)
    cm[:1 + len(vj)] = 1.0
    cmf = np.concatenate(
        [cm.reshape(njt, 128).T, np.tile(null_key, 2).reshape(128, 1)], axis=1)

    gs = slice(g * DG, (g + 1) * DG)
    return {
        "xT": np.ascontiguousarray(xc.T).astype(NPBF),
        "cxT": np.ascontiguousarray(cxc.T).astype(NPBF),
        "wq": np.ascontiguousarray(Wq[:, gs]).astype(NPBF),
        "wk": np.ascontiguousarray(Wkv[:, gs]).astype(NPBF),
        "wv": np.ascontiguousarray(
            Wkv[:, DIM + g * DG: DIM + (g + 1) * DG]).astype(NPBF),
        "wo": np.ascontiguousarray(Wo[gs, :]).astype(NPBF),
        "cmf": np.ascontiguousarray(cmf),
        "nv": np.tile(null_value, HG).reshape(1, HG * DH).astype(NPBF),
    }


def kernel(x, context, mask, context_mask, Wq, Wkv, Wo, bo, null_key, null_value):
    global LAST_RESULTS
    inputs = {
        "x": x, "context": context, "mask": mask, "context_mask": context_mask,
        "Wq": Wq, "Wkv": Wkv, "Wo": Wo, "bo": bo,
        "null_key": null_key, "null_value": null_value,
    }
    mask = np.asarray(mask)
    context_mask = np.asarray(context_mask)
    nip = _pad128(int(mask.sum(1).max()))
    njp = _pad128(int(context_mask.sum(1).max()) + 1)

    key = (nip, njp)
    if key not in _CACHE:
        _CACHE[key] = _build(nip, njp)
        _CACHE["nc"] = _CACHE[key]   # convenience handle for test.py
    nc = _CACHE[key]
    in_maps = [_core_inputs(inputs, core, nip, njp) for core in range(8)]
    res = bass_utils.run_bass_kernel_spmd(nc, in_maps, core_ids=list(range(8)))
    LAST_RESULTS = res

    x_np = np.asarray(x, np.float32)
    ctx_np = np.asarray(context, np.float32)
    Wkv_np = np.asarray(Wkv, np.float32)
    Wo_np = np.asarray(Wo, np.float32)
    bo_np = np.asarray(bo, np.float32)
    nv_np = np.asarray(null_value, np.float32)

    out = np.empty((B, N, DIM), np.float32)
    for b in range(B):
        vi = np.flatnonzero(mask[b])
        dev = (np.asarray(res.results[2 * b]["out"], np.float32)
               + np.asarray(res.results[2 * b + 1]["out"], np.float32))
        out[b][vi] = dev[:len(vi)] + bo_np
        if len(vi) < N:
            # masked queries: uniform attention over [null, all keys]
            vsum = ctx_np[b].sum(0) @ Wkv_np[:, DIM:] + np.tile(nv_np, HEADS)
            urow = (vsum / (M + 1)) @ Wo_np + bo_np
            out[b][~mask[b]] = urow
    return out
